# revision 1
# baseline (speedup 1.0000x reference)
"""ClusterisedLinearNetwork Trainium2 kernel.

Math: per token t (N=262144):
  enc[t] = NeRF positional encoding of X[t] (120 dims, 10 freqs x sin/cos x 6)
  out_all[t] = enc[t] @ W.T  -> [256 clusters, 3]
  rgb[t, j] = sum_k weights[k,t] * out_all[t, cluster_ids[t,k], j]

Device formulation (avoids materializing the 768-wide dense output):
  Q[c, t]  = sum_k weights[k,t] * [cluster_ids[t,k] == c]   (routing matrix, host-densified)
  M_j[d,t] = sum_c W[(c,j), d] * Q[c, t]                     (TensorE matmul)
  rgb[j,t] = sum_d enc[d,t] * M_j[d,t]                       (DVE mult + ones-matmul reduce)

Identical FLOP count to the dense matmul (768x120 per token), but the gather/
weighted-sum is absorbed into the contraction, so no 768-wide per-token
selection pass is needed.

Sharding: data-parallel over 8 NeuronCores along the token axis; W replicated.

Note from optimization session 2026-08-08: alternatives evaluated on HW —
cluster-sorted MoE segmentation (tiny [120,3] stationaries, 0.55-0.71 ns/col
with ~300-670 ns/matmul overhead from stationary swaps + 32-col PE tile mode)
and a dense+sorted hybrid (measured 209us vs 136us for this kernel with a
noise-robust large-reps protocol). This dense formulation runs within ~10% of
its PE streaming floor (294912 cols/core @ 2.4 GHz = 123us) and remained the
fastest correct design.
"""
import sys
sys.path.insert(0, '/opt/trn_rl_repo')
import numpy as np

N_TOK = 262144
N_CORES = 8
NPC = N_TOK // N_CORES          # 32768 tokens per core
C = 256                          # clusters
F = 10                           # freq bands
D = 120                          # encoding dim
T = 512                          # tokens per inner group (one PSUM bank fp32)
TM = 2048                        # tokens per macro group (DMA/ACT batching)
NG = NPC // TM                   # macro groups per core (16)
GPM = TM // T                    # inner groups per macro (4)

DT_Q = None                      # set in _build: mybir dtype for Q/WQ operands
USE_BF16 = True

_compiled = None


def _host_prep_shared(X, W, weights, cluster_ids):
    """Host-side input conditioning (layout + routing densification)."""
    X = np.asarray(X, dtype=np.float32)
    W = np.asarray(W, dtype=np.float32)
    weights = np.asarray(weights, dtype=np.float32)
    ids = np.asarray(cluster_ids).astype(np.int64)

    # --- row order for the encoding axis (d'): rows 0..59 sin(2^f x_d), 60..119 cos ---
    r = np.arange(D)
    f_arr = np.where(r < 60, r // 6, (r - 60) // 6)
    d_arr = np.where(r < 60, r % 6, (r - 60) % 6)
    phase = np.where(r < 60, 0.0, np.pi / 2)
    # original enc column for permuted row r: f*12 + s*6 + d
    s_arr = (r >= 60).astype(np.int64)
    perm = f_arr * 12 + s_arr * 6 + d_arr

    # --- Y: range-reduced sin arguments, [120, N] fp32 ---
    # xb = x_d * 2^f (+ pi/2 for cos rows); y = xb mod 2pi -> [-pi, pi]
    Xd = X[:, d_arr].astype(np.float64).T          # [120, N]
    Y = Xd * (2.0 ** f_arr)[:, None] + phase[:, None]
    Y -= np.round(Y / (2 * np.pi)) * (2 * np.pi)
    Y = np.round(Y / np.pi * 32767.0).astype(np.int16)

    # --- Q: weighted one-hot routing matrix [256, N] ---
    Q = np.zeros((C, N_TOK), np.float32)
    t_idx = np.arange(N_TOK)
    for k in range(3):
        np.add.at(Q, (ids[:, k], t_idx), weights[k, :])
    # masked tokens (first 3 coords all exactly -1) produce zero output
    mask = np.all(X[:, :3] == -1.0, axis=-1)
    if mask.any():
        Q[:, mask] = 0.0

    # --- WQ: lhsT blocks [c'=128, d=120] for (j, h) ---
    # WQcat[c', (j*2+h)*120 + d'] = W[3*(128*h + c') + j, perm[d']]
    Wp = W[:, perm]                                # [768, 120]
    WQcat = np.empty((128, 6 * D), np.float32)
    for j in range(3):
        for h in range(2):
            blk = Wp[3 * (128 * h + np.arange(128)) + j, :]   # [128, 120]
            WQcat[:, (j * 2 + h) * D:(j * 2 + h + 1) * D] = blk
    return Y, Q, WQcat


def _build(reps=1):
    """Compile the per-core Bass kernel (SPMD; same program all 8 cores)."""
    global _compiled
    if _compiled is not None and _compiled[0] == reps:
        return _compiled[1]
    from concourse import bacc, tile, mybir
    from contextlib import ExitStack

    dtq = mybir.dt.bfloat16 if USE_BF16 else mybir.dt.float32
    f32 = mybir.dt.float32

    nc = bacc.Bacc("TRN2", target_bir_lowering=False, debug=False,
                   num_devices=N_CORES)

    Ycat = nc.dram_tensor("Ycat", [D, NPC], mybir.dt.int16, kind="ExternalInput")
    Qcat = nc.dram_tensor("Qcat", [128, 2 * NPC], dtq, kind="ExternalInput")
    WQc = nc.dram_tensor("WQc", [128, 6 * D], dtq, kind="ExternalInput")
    rgbh = nc.dram_tensor("rgbh", [3, NPC], f32, kind="ExternalOutput")

    with tile.TileContext(nc) as tc:
        with tc.tile_pool(name="const", bufs=1) as cpool, \
             tc.tile_pool(name="ysl", bufs=3) as ypool, \
             tc.tile_pool(name="enc", bufs=3) as epool, \
             tc.tile_pool(name="q", bufs=3) as qpool, \
             tc.tile_pool(name="p", bufs=3) as ppool, \
             tc.tile_pool(name="rgb", bufs=2) as rpool, \
             tc.tile_pool(name="mall", bufs=2, space="PSUM") as mpool, \
             tc.tile_pool(name="rgbp", bufs=2, space="PSUM") as rppool:

            wq = cpool.tile([128, 6 * D], dtq)
            nc.sync.dma_start(wq[:], WQc.ap())
            bf16 = mybir.dt.bfloat16
            ones_t = nc.const_aps.tensor(1.0, (D, 1), bf16)

            rep_ctx = ExitStack()
            if reps > 1:
                rep_ctx.enter_context(tc.For_i(0, reps, 1))
            NGRP = NG * GPM
            state = {}          # per-group carried tiles for 1-group delay
            rgb66_by_macro = {}
            for gg in range(NGRP + 1):
                if gg < NGRP:
                    m, g = divmod(gg, GPM)
                    moff = m * TM
                    goff = g * T
                    if g == 0:
                        y_sb = ypool.tile([D, TM], mybir.dt.int16, tag="y")
                        encT = epool.tile([D, TM], f32, tag="enc")
                        q_sb = qpool.tile([128, 2, TM], dtq, tag="q")
                        nsin = GPM if m == 0 else 1
                        for u in range(nsin):
                            usl = slice(u * TM // nsin, (u + 1) * TM // nsin)
                            nc.sync.dma_start(y_sb[:, usl],
                                              Ycat.ap()[:, moff + u * TM // nsin:
                                                        moff + (u + 1) * TM // nsin])
                            nc.scalar.activation(encT[:, usl], y_sb[:, usl],
                                                 mybir.ActivationFunctionType.Sin,
                                                 bias=0.0,
                                                 scale=float(np.pi / 32767.0))
                        rgb66_by_macro[m] = rpool.tile([66, TM], f32, tag="rgb66", name="rgb66")
                        state["enc"], state["q"] = encT, q_sb
                    encT, q_sb = state["enc"], state["q"]
                    if g == 0:
                        for h in range(2):
                            for u in range(2):
                                nc.sync.dma_start(
                                    q_sb[:, h, u * TM // 2:(u + 1) * TM // 2],
                                    Qcat.ap()[:, h * NPC + moff + u * TM // 2:
                                              h * NPC + moff + (u + 1) * TM // 2])
                    mall = mpool.tile([128, 3 * T], f32, tag="mall")
                    for j in range(3):
                        for h in range(2):
                            nc.tensor.matmul(
                                mall[0:D, j * T:(j + 1) * T],
                                lhsT=wq[:, (j * 2 + h) * D:(j * 2 + h + 1) * D],
                                rhs=q_sb[:, h, goff:goff + T],
                                start=(h == 0), stop=(h == 1))
                    p_all = ppool.tile([D, 3 * T], bf16, tag="p")
                    nc.vector.tensor_tensor(
                        out=p_all[:].rearrange('p (j t) -> p j t', j=3),
                        in0=mall[0:D, :].rearrange('p (j t) -> p j t', j=3),
                        in1=encT[:, None, goff:goff + T].to_broadcast([D, 3, T]),
                        op=mybir.AluOpType.mult)
                    state[gg] = p_all
                # delayed-by-one reduce + evacuation
                pg = gg - 1
                if pg >= 0:
                    pm, pgr = divmod(pg, GPM)
                    pgoff = pgr * T
                    p_prev = state.pop(pg)
                    rgbp = rppool.tile([66, T], f32, tag="rgbp")
                    for j in range(3):
                        nc.tensor.matmul(
                            rgbp[32 * j:32 * j + 1, :],
                            lhsT=ones_t,
                            rhs=p_prev[:, j * T:(j + 1) * T],
                            start=True, stop=True)
                    rgb66p = rgb66_by_macro[pm]
                    nc.scalar.copy(rgb66p[:, pgoff:pgoff + T], rgbp[:])
                    if pgr == GPM - 1:
                        nc.sync.dma_start(rgbh.ap()[:, pm * TM:(pm + 1) * TM],
                                          rgb66p[0:66:32, :])
                        del rgb66_by_macro[pm]
            rep_ctx.close()

    nc.compile()
    _compiled = (reps, nc)
    return nc


def kernel(X, W, weights, cluster_ids, _want_trace=False, _trace_kwargs=None):
    from concourse import bass_utils
    import ml_dtypes

    nc = _build()
    Y, Q, WQcat = _host_prep_shared(X, W, weights, cluster_ids)

    np_q = ml_dtypes.bfloat16 if USE_BF16 else np.float32
    WQc_np = WQcat.astype(np_q)
    in_maps = []
    for c in range(N_CORES):
        sl = slice(c * NPC, (c + 1) * NPC)
        Qc = Q[:, sl]                              # [256, NPC]
        Qcat_np = np.concatenate([Qc[0:128, :], Qc[128:256, :]],
                                 axis=1).astype(np_q)   # [128, 2*NPC]
        in_maps.append({
            "Ycat": np.ascontiguousarray(Y[:, sl]),
            "Qcat": Qcat_np,
            "WQc": WQc_np,
        })

    kw = {}
    if _want_trace:
        kw = dict(trace=True, **(_trace_kwargs or {}))
    res = bass_utils.run_bass_kernel_spmd(nc, in_maps,
                                          core_ids=list(range(N_CORES)), **kw)
    out = np.empty((N_TOK, 3), np.float32)
    for c in range(N_CORES):
        out[c * NPC:(c + 1) * NPC, :] = np.asarray(res.results[c]["rgbh"]).T
    if _want_trace:
        return out, res
    return out



# revision 2
# speedup vs baseline: 1.0341x; 1.0341x over previous
"""ClusterisedLinearNetwork Trainium2 kernel.

Math per token t (N=262144):
  enc[t] = NeRF positional encoding of X[t] (120 dims, 10 freqs x sin/cos x 6)
  out_all[t] = enc[t] @ W.T  -> [256 clusters, 3]
  rgb[t, j] = sum_k weights[k,t] * out_all[t, cluster_ids[t,k], j]

Device formulation (dense; routing absorbed into the contraction):
  Q[c, t]   = sum_k weights[k,t] * [cluster_ids[t,k] == c]   (host-densified)
  M_j[d, t] = sum_c W[(c,j), d] * Q[c, t]                    (PE matmul, bf16)
  p         = M_j ⊙ enc                                      (DVE / ACT+DVE)
  rgb[j, t] = sum_d p[d, (j,t)]                              (PE ones-matmuls,
                                                              col-tiled strips)

Sharding: data-parallel over 8 NeuronCores along the token axis; W replicated.

Optimization session 2026-08-10 (measured, low-noise device-resident-input
protocol; v1 = previous session's kernel = 141.7us on the same protocol):
  - enc precomputed on host in bf16 (frees ACT from Sin; same DMA bytes)
  - macro = 1024 tokens with a j-outer loop: 6 LDWEIGHTS per 1024 tokens
    (LDWEIGHTS measured fully serial with matmuls here: the main-matmul-only
    floor was 100.9us = 12 MM x 518cyc + 6 LDW x 120cols per macro)
  - stationaries zero-padded to 128 cols so NumWeights==128 triggers the
    compiler's FWL fast weight load (2x for bf16)
  - the enc⊙M multiply is split: j0 read directly from PSUM on DVE (1x mode),
    j1/j2 evacuated PSUM->SBUF by ScalarE then multiplied on DVE in 2x mode —
    keeps both DVE (0.96GHz) and ACT (1.2GHz) under the PE roofline
  - reduce matmuls auto-col-tile to strips 0/1/2 (out partitions 32j)
  - fp8 (DoubleRow) evaluated numerically and rejected: W/Q/p e4m3
    quantization each exceed the 2e-2 gate (random-sign sums don't average
    away relative quantization error)
"""
import sys
sys.path.insert(0, '/opt/trn_rl_repo')
import numpy as np

N_TOK = 262144
N_CORES = 8
NPC = N_TOK // N_CORES          # 32768 tokens per core
C = 256                          # clusters
F = 10                           # freq bands
D = 120                          # encoding dim
T = 512                          # tokens per PSUM-bank group
TM = 1024                        # tokens per macro (j-outer stationary reuse)
NMAC = NPC // TM                 # macros per core (32)

_compiled = None


def _host_prep_shared(X, W, weights, cluster_ids):
    """Host-side input conditioning (layout + routing densification)."""
    X = np.asarray(X, dtype=np.float32)
    W = np.asarray(W, dtype=np.float32)
    weights = np.asarray(weights, dtype=np.float32)
    ids = np.asarray(cluster_ids).astype(np.int64)

    # row order for the encoding axis: rows 0..59 sin(2^f x_d), 60..119 cos
    r = np.arange(D)
    f_arr = np.where(r < 60, r // 6, (r - 60) // 6)
    d_arr = np.where(r < 60, r % 6, (r - 60) % 6)
    phase = np.where(r < 60, 0.0, np.pi / 2)
    s_arr = (r >= 60).astype(np.int64)
    perm = f_arr * 12 + s_arr * 6 + d_arr   # original enc column for row r

    # E: encoding, [120, N] (cast to bf16 at shipping)
    Xd = X[:, d_arr].astype(np.float64).T
    E = np.sin(Xd * (2.0 ** f_arr)[:, None] + phase[:, None]).astype(np.float32)

    # Q: weighted one-hot routing matrix [256, N]
    Q = np.zeros((C, N_TOK), np.float32)
    t_idx = np.arange(N_TOK)
    for k in range(3):
        np.add.at(Q, (ids[:, k], t_idx), weights[k, :])
    mask = np.all(X[:, :3] == -1.0, axis=-1)
    if mask.any():
        Q[:, mask] = 0.0

    # WQ: lhsT blocks [c'=128, d=128] for (j, h); cols 120..127 zero-padded
    # so NumWeights==128 triggers FWL (2x faster LDWEIGHTS for bf16)
    Wp = W[:, perm]                                        # [768, 120]
    WQcat = np.zeros((128, 6 * 128), np.float32)
    for j in range(3):
        for h in range(2):
            blk = Wp[3 * (128 * h + np.arange(128)) + j, :]
            WQcat[:, (j * 2 + h) * 128:(j * 2 + h) * 128 + D] = blk
    return E, Q, WQcat


def _build(reps=1):
    """Compile the per-core Bass kernel (SPMD; same program all 8 cores)."""
    global _compiled
    if _compiled is not None and _compiled[0] == reps:
        return _compiled[1]
    from concourse import bacc, tile, mybir
    from contextlib import ExitStack

    bf16 = mybir.dt.bfloat16
    f32 = mybir.dt.float32

    nc = bacc.Bacc("TRN2", target_bir_lowering=False, debug=False,
                   num_devices=N_CORES)

    Ecat = nc.dram_tensor("Ecat", [D, NPC], bf16, kind="ExternalInput")
    Qcat = nc.dram_tensor("Qcat", [128, 2 * NPC], bf16, kind="ExternalInput")
    WQc = nc.dram_tensor("WQc", [128, 6 * 128], bf16, kind="ExternalInput")
    rgbh = nc.dram_tensor("rgbh", [3, NPC], f32, kind="ExternalOutput")

    with tile.TileContext(nc) as tc:
        with tc.tile_pool(name="const", bufs=1) as cpool, \
             tc.tile_pool(name="q", bufs=3) as qpool, \
             tc.tile_pool(name="enc", bufs=3) as epool, \
             tc.tile_pool(name="p", bufs=2) as ppool, \
             tc.tile_pool(name="mc", bufs=3) as mcpool, \
             tc.tile_pool(name="rgb", bufs=2) as rpool, \
             tc.tile_pool(name="mall", bufs=3, space="PSUM") as mpool, \
             tc.tile_pool(name="rgbp", bufs=2, space="PSUM") as rppool:

            wq = cpool.tile([128, 6 * 128], bf16)
            nc.sync.dma_start(wq[:], WQc.ap())
            ones_c = cpool.tile([D, 1], bf16)
            nc.vector.memset(ones_c[:], 1.0)

            rep_ctx = ExitStack()
            if reps > 1:
                rep_ctx.enter_context(tc.For_i(0, reps, 1))
            state = {}
            for mi in range(NMAC + 1):
                if mi < NMAC:
                    moff = mi * TM
                    q_sb = qpool.tile([128, 2 * TM], bf16, tag="q")
                    nc.sync.dma_start(q_sb[:],
                                      Qcat.ap()[:, 2 * moff:2 * moff + 2 * TM])
                    encm = epool.tile([D, TM], bf16, tag="enc")
                    nc.sync.dma_start(encm[:], Ecat.ap()[:, moff:moff + TM])
                    p = ppool.tile([D, 3, TM], bf16, tag="p")
                    for j in range(3):
                        mall = mpool.tile([128, TM], f32, tag="mall")
                        for h in range(2):
                            for g in range(TM // T):
                                nc.tensor.matmul(
                                    mall[:, g * T:(g + 1) * T],
                                    lhsT=wq[:, (j * 2 + h) * 128:
                                            (j * 2 + h + 1) * 128],
                                    rhs=q_sb[:, h * TM + g * T:
                                             h * TM + (g + 1) * T],
                                    start=(h == 0), stop=(h == 1))
                        if j == 0:
                            # DVE reads PSUM directly (1x mode)
                            nc.vector.tensor_tensor(
                                out=p[:, j, :], in0=mall[0:D, :], in1=encm[:],
                                op=mybir.AluOpType.mult)
                        else:
                            # ScalarE evacuates PSUM; DVE multiplies in 2x mode
                            mcj = mcpool.tile([D, TM], bf16, tag="mc")
                            nc.scalar.copy(mcj[:], mall[0:D, :])
                            nc.vector.tensor_tensor(
                                out=p[:, j, :], in0=mcj[:], in1=encm[:],
                                op=mybir.AluOpType.mult)
                    state[mi] = p
                # delayed-by-one-macro reduce + output (keeps PE dense)
                pm = mi - 1
                if pm < 0:
                    continue
                p_prev = state.pop(pm)
                rgb66 = rpool.tile([66, TM], f32, tag="rgb66")
                for g in range(TM // T):
                    rgbp = rppool.tile([66, T], f32, tag="rgbp")
                    for j in range(3):
                        nc.tensor.matmul(
                            rgbp[32 * j:32 * j + 1, :],
                            lhsT=ones_c[:],
                            rhs=p_prev[:, j, g * T:(g + 1) * T],
                            start=True, stop=True)
                    nc.scalar.copy(rgb66[:, g * T:(g + 1) * T], rgbp[:])
                nc.sync.dma_start(rgbh.ap()[:, pm * TM:(pm + 1) * TM],
                                  rgb66[0:66:32, :])
            rep_ctx.close()

    nc.compile()
    _compiled = (reps, nc)
    return nc


def _make_in_maps(E, Q, WQcat):
    import ml_dtypes
    bf = ml_dtypes.bfloat16
    WQ_np = WQcat.astype(bf)
    in_maps = []
    for c in range(N_CORES):
        sl = slice(c * NPC, (c + 1) * NPC)
        Qc = Q[:, sl].astype(bf)                      # [256, NPC]
        # macro-major, h-interleaved: [m][h=0 block TM][h=1 block TM]
        Qm = np.empty((128, 2 * NPC), bf)
        for m in range(NMAC):
            Qm[:, 2 * m * TM:(2 * m + 1) * TM] = Qc[0:128, m * TM:(m + 1) * TM]
            Qm[:, (2 * m + 1) * TM:(2 * m + 2) * TM] = \
                Qc[128:256, m * TM:(m + 1) * TM]
        in_maps.append({
            "Ecat": np.ascontiguousarray(E[:, sl]).astype(bf),
            "Qcat": Qm,
            "WQc": WQ_np,
        })
    return in_maps


def kernel(X, W, weights, cluster_ids, _want_trace=False, _trace_kwargs=None):
    from concourse import bass_utils

    nc = _build()
    E, Q, WQcat = _host_prep_shared(X, W, weights, cluster_ids)
    in_maps = _make_in_maps(E, Q, WQcat)

    kw = {}
    if _want_trace:
        kw = dict(trace=True, **(_trace_kwargs or {}))
    res = bass_utils.run_bass_kernel_spmd(nc, in_maps,
                                          core_ids=list(range(N_CORES)), **kw)
    out = np.empty((N_TOK, 3), np.float32)
    for c in range(N_CORES):
        out[c * NPC:(c + 1) * NPC, :] = np.asarray(res.results[c]["rgbh"]).T
    if _want_trace:
        return out, res
    return out


# revision 4
# speedup vs baseline: 1.1331x; 1.0957x over previous
"""ClusterisedLinearNetwork Trainium2 kernel.

Math per token t (N=262144):
  enc[t] = NeRF positional encoding of X[t] (120 dims, 10 freqs x sin/cos x 6)
  out_all[t] = enc[t] @ W.T  -> [256 clusters, 3]
  rgb[t, j] = sum_k weights[k,t] * out_all[t, cluster_ids[t,k], j]

Device formulation (dense; routing absorbed into the contraction):
  Q[c, t]   = sum_k weights[k,t] * [cluster_ids[t,k] == c]   (host-densified)
  M_j[d, t] = sum_c W[(c,j), d] * Q[c, t]                    (PE matmul, bf16)
  p         = M_j ⊙ enc                                      (DVE / ACT+DVE)
  rgb[j, t] = sum_d p[d, (j,t)]                              (PE ones-matmuls,
                                                              col-tiled strips)

Sharding: data-parallel over 8 NeuronCores along the token axis; W replicated.

Optimization session 2026-08-10 (measured, low-noise device-resident-input
protocol; v1 = previous session's kernel = 141.7us on the same protocol):
  - enc precomputed on host in bf16 (frees ACT from Sin; same DMA bytes)
  - macro = 1024 tokens with a j-outer loop: 6 LDWEIGHTS per 1024 tokens
    (LDWEIGHTS measured fully serial with matmuls here: the main-matmul-only
    floor was 100.9us = 12 MM x 518cyc + 6 LDW x 120cols per macro)
  - stationaries zero-padded to 128 cols so NumWeights==128 triggers the
    compiler's FWL fast weight load (2x for bf16)
  - the enc⊙M multiply is split: j0 read directly from PSUM on DVE (1x mode),
    j1/j2 evacuated PSUM->SBUF by ScalarE then multiplied on DVE in 2x mode —
    keeps both DVE (0.96GHz) and ACT (1.2GHz) under the PE roofline
  - reduce matmuls auto-col-tile to strips 0/1/2 (out partitions 32j)
  - fp8 (DoubleRow) evaluated numerically and rejected: W/Q/p e4m3
    quantization each exceed the 2e-2 gate (random-sign sums don't average
    away relative quantization error)
"""
import sys
sys.path.insert(0, '/opt/trn_rl_repo')
import numpy as np

N_TOK = 262144
N_CORES = 8
NPC = N_TOK // N_CORES          # 32768 tokens per core
C = 256                          # clusters
F = 10                           # freq bands
D = 120                          # encoding dim
T = 512                          # tokens per PSUM-bank group
TM = 1024                        # tokens per macro (j-outer stationary reuse)
NMAC = NPC // TM                 # macros per core (32)

_compiled = None


def _host_prep_shared(X, W, weights, cluster_ids):
    """Host-side input conditioning (layout + routing densification)."""
    X = np.asarray(X, dtype=np.float32)
    W = np.asarray(W, dtype=np.float32)
    weights = np.asarray(weights, dtype=np.float32)
    ids = np.asarray(cluster_ids).astype(np.int64)

    # row order for the encoding axis: rows 0..59 sin(2^f x_d), 60..119 cos
    r = np.arange(D)
    f_arr = np.where(r < 60, r // 6, (r - 60) // 6)
    d_arr = np.where(r < 60, r % 6, (r - 60) % 6)
    phase = np.where(r < 60, 0.0, np.pi / 2)
    s_arr = (r >= 60).astype(np.int64)
    perm = f_arr * 12 + s_arr * 6 + d_arr   # original enc column for row r

    # E: encoding, [120, N] (cast to bf16 at shipping)
    Xd = X[:, d_arr].astype(np.float64).T
    E = np.sin(Xd * (2.0 ** f_arr)[:, None] + phase[:, None]).astype(np.float32)

    # Q: weighted one-hot routing matrix [256, N]
    Q = np.zeros((C, N_TOK), np.float32)
    t_idx = np.arange(N_TOK)
    for k in range(3):
        np.add.at(Q, (ids[:, k], t_idx), weights[k, :])
    mask = np.all(X[:, :3] == -1.0, axis=-1)
    if mask.any():
        Q[:, mask] = 0.0

    # WQ: lhsT blocks [c'=128, d=128] for (j, h); cols 120..127 zero-padded
    # so NumWeights==128 triggers FWL (2x faster LDWEIGHTS for bf16)
    Wp = W[:, perm]                                        # [768, 120]
    WQcat = np.zeros((128, 6 * 128), np.float32)
    for j in range(3):
        for h in range(2):
            blk = Wp[3 * (128 * h + np.arange(128)) + j, :]
            WQcat[:, (j * 2 + h) * 128:(j * 2 + h) * 128 + D] = blk
    return E, Q, WQcat


def _build(reps=1):
    """Compile the per-core Bass kernel (SPMD; same program all 8 cores)."""
    global _compiled
    if _compiled is not None and _compiled[0] == reps:
        return _compiled[1]
    from concourse import bacc, tile, mybir
    from contextlib import ExitStack

    bf16 = mybir.dt.bfloat16
    f32 = mybir.dt.float32

    nc = bacc.Bacc("TRN2", target_bir_lowering=False, debug=False,
                   num_devices=N_CORES)

    Ecat = nc.dram_tensor("Ecat", [D, NPC], bf16, kind="ExternalInput")
    Qcat = nc.dram_tensor("Qcat", [128, 2 * NPC], bf16, kind="ExternalInput")
    WQc = nc.dram_tensor("WQc", [128, 6 * 128], bf16, kind="ExternalInput")
    rgbh = nc.dram_tensor("rgbh", [3, NPC], f32, kind="ExternalOutput")

    with tile.TileContext(nc) as tc:
        with tc.tile_pool(name="const", bufs=1) as cpool, \
             tc.tile_pool(name="q", bufs=4) as qpool, \
             tc.tile_pool(name="enc", bufs=4) as epool, \
             tc.tile_pool(name="p", bufs=3) as ppool, \
             tc.tile_pool(name="mc", bufs=4) as mcpool, \
             tc.tile_pool(name="rgb", bufs=3) as rpool, \
             tc.tile_pool(name="mall", bufs=3, space="PSUM") as mpool, \
             tc.tile_pool(name="rgbp", bufs=2, space="PSUM") as rppool:

            wq = cpool.tile([128, 6 * 128], bf16)
            nc.sync.dma_start(wq[:], WQc.ap())
            ones_c = cpool.tile([D, 1], bf16)
            nc.vector.memset(ones_c[:], 1.0)

            rep_ctx = ExitStack()
            if reps > 1:
                rep_ctx.enter_context(tc.For_i(0, reps, 1))
            state = {}
            for mi in range(NMAC + 1):
                if mi < NMAC:
                    moff = mi * TM
                    q_sb = qpool.tile([128, 2 * TM], bf16, tag="q")
                    nc.sync.dma_start(q_sb[:],
                                      Qcat.ap()[:, 2 * moff:2 * moff + 2 * TM])
                    encm = epool.tile([D, TM], bf16, tag="enc")
                    nc.sync.dma_start(encm[:], Ecat.ap()[:, moff:moff + TM])
                    p = ppool.tile([D, 3, TM], bf16, tag="p")
                    for j in range(3):
                        mall = mpool.tile([128, TM], f32, tag="mall")
                        for h in range(2):
                            for g in range(TM // T):
                                nc.tensor.matmul(
                                    mall[:, g * T:(g + 1) * T],
                                    lhsT=wq[:, (j * 2 + h) * 128:
                                            (j * 2 + h + 1) * 128],
                                    rhs=q_sb[:, h * TM + g * T:
                                             h * TM + (g + 1) * T],
                                    start=(h == 0), stop=(h == 1))
                        if j == 2:
                            # DVE reads PSUM directly (1x mode). Direct path
                            # on the LAST j: its slow TT holds the PSUM bank
                            # longest, and j2's bank isn't needed again until
                            # the end of the next macro (max reuse slack).
                            nc.vector.tensor_tensor(
                                out=p[:, j, :], in0=mall[0:D, :], in1=encm[:],
                                op=mybir.AluOpType.mult)
                        else:
                            # ScalarE evacuates PSUM; DVE multiplies in 2x mode
                            mcj = mcpool.tile([D, TM], bf16, tag="mc")
                            nc.scalar.copy(mcj[:], mall[0:D, :])
                            nc.vector.tensor_tensor(
                                out=p[:, j, :], in0=mcj[:], in1=encm[:],
                                op=mybir.AluOpType.mult)
                    state[mi] = p
                # delayed-by-one-macro reduce + output (keeps PE dense)
                pm = mi - 1
                if pm < 0:
                    continue
                p_prev = state.pop(pm)
                rgb66 = rpool.tile([66, TM], f32, tag="rgb66")
                for g in range(TM // T):
                    rgbp = rppool.tile([66, T], f32, tag="rgbp")
                    for j in range(3):
                        nc.tensor.matmul(
                            rgbp[32 * j:32 * j + 1, :],
                            lhsT=ones_c[:],
                            rhs=p_prev[:, j, g * T:(g + 1) * T],
                            start=True, stop=True)
                    nc.scalar.copy(rgb66[:, g * T:(g + 1) * T], rgbp[:])
                nc.sync.dma_start(rgbh.ap()[:, pm * TM:(pm + 1) * TM],
                                  rgb66[0:66:32, :])
            rep_ctx.close()

    nc.compile()
    _compiled = (reps, nc)
    return nc


def _make_in_maps(E, Q, WQcat):
    import ml_dtypes
    bf = ml_dtypes.bfloat16
    WQ_np = WQcat.astype(bf)
    in_maps = []
    for c in range(N_CORES):
        sl = slice(c * NPC, (c + 1) * NPC)
        Qc = Q[:, sl].astype(bf)                      # [256, NPC]
        # macro-major, h-interleaved: [m][h=0 block TM][h=1 block TM]
        Qm = np.empty((128, 2 * NPC), bf)
        for m in range(NMAC):
            Qm[:, 2 * m * TM:(2 * m + 1) * TM] = Qc[0:128, m * TM:(m + 1) * TM]
            Qm[:, (2 * m + 1) * TM:(2 * m + 2) * TM] = \
                Qc[128:256, m * TM:(m + 1) * TM]
        in_maps.append({
            "Ecat": np.ascontiguousarray(E[:, sl]).astype(bf),
            "Qcat": Qm,
            "WQc": WQ_np,
        })
    return in_maps


def kernel(X, W, weights, cluster_ids, _want_trace=False, _trace_kwargs=None):
    from concourse import bass_utils

    nc = _build()
    E, Q, WQcat = _host_prep_shared(X, W, weights, cluster_ids)
    in_maps = _make_in_maps(E, Q, WQcat)

    kw = {}
    if _want_trace:
        kw = dict(trace=True, **(_trace_kwargs or {}))
    res = bass_utils.run_bass_kernel_spmd(nc, in_maps,
                                          core_ids=list(range(N_CORES)), **kw)
    out = np.empty((N_TOK, 3), np.float32)
    for c in range(N_CORES):
        out[c * NPC:(c + 1) * NPC, :] = np.asarray(res.results[c]["rgbh"]).T
    if _want_trace:
        return out, res
    return out
